# revision 67
# baseline (speedup 1.0000x reference)
"""Trainium2 Bass kernel for the sparse_attention PoC block.

Reference (per batch item):
  qkv = x @ qkv_w.T; q,k,v heads [H, N, D]
  attn = (q @ k.T) * scale; block edits: attn[:S1, S2:] = attn[:S1, S1:S2]
  (pre-bias copy), -100 bias on [:S1, S1:S2], [S1:S2, S2:], [S2:, S1:S2];
  softmax; attn @ v; proj + bias.

Distribution: data-parallel over batch B=64 across 8 cores (8 items per
core, weights replicated, no collectives).

Per-core design (all matmuls bf16, fp32 PSUM):
  - x and the weights are transposed on the PE (f32 in, 2 cyc/row,
    identity matmul); the PSUM drain does the f32->bf16 cast, so there
    are no standalone cast instructions.  (DMA XBAR transposes were
    tried and are FASTER per op, but racy: concurrent dma_start_transpose
    dispatch from two queues corrupts data nondeterministically.)
  - Batches processed in groups of 2; q/k matmuls fuse both batches into
    one rhs [128c, 2, 236] (472-column streams).
  - k uses a 256-wide EXTENDED key axis: 0:216 real tokens, 216:236 lang
    keys (the pre-bias "copy" edit), 236:256 real aux keys.  Rank-3 bias
    rows (u1/u2/u3 on the k side x w1/w2/w3 on the q side, contraction
    rows 96:99) route each query group to the right slots.
  - scores sT[j, n] = k_ext.T @ q_ext per (head, key-tile); softmax
    without max-subtraction (suppressed entries underflow exp to 0);
    exp on ScalarE with 1/sqrt(D) folded in.
  - v packed [128j, H, D+1] ([v | ones]); attn@v gives oT [D+1, n] with
    a denominator row; normalize via fast reciprocal + gpsimd
    partition_broadcast; proj per head (K=96, +bias row on head 0).
  - SOFTWARE PIPELINE: group g's qk-chunk matmuls are issued BETWEEN the
    score and attn@v matmuls of group g-1's attention so the PE never
    stalls waiting on ScalarE's exp (PE p-state ramp: idle gaps drop the
    clock 2.4GHz -> 1.2GHz).  v/proj/x-prep fill the group tail; group
    0's qk chunks interleave with the weight-prep transposes so the PE
    starts ~13us into the kernel.

Engine split: PE matmuls+transposes; DVE/ScalarE split the psum drains
(ScalarE also exp); Pool(gpsimd) sbuf->sbuf dup/bias-rows/broadcast (no
PSUM port - and it is ~3x slower per element than DVE, so only
off-critical-path work goes there); SP = input DMA only, out-DMAs ride
the Activation hwdge queue behind their drains.

Partition rule (walrus): compute-engine APs start at partition 0/32/64/96
(max 128/32/64/32); matmul operands start at partition 0.
"""

import numpy as np

B, N, C = 64, 236, 768
H, D = 8, 96
S1, S2 = 196, 216
BIAS = 100.0
SCALE = D ** -0.5
BIAS_RAW = BIAS / SCALE  # applied on raw (pre-scale) scores

N_CORES = 8
B_LOC = B // N_CORES

NT = [(0, 128), (128, 108)]  # token tiles
NCH = C // 128  # 6 contraction chunks over C
KEXT = 128  # score contraction: 96 head dims + bias rows
NK = 256  # extended key axis


def part_cap(s):
    return 128 if s == 0 else 64 if s == 64 else 32


def part_pieces2(s1, s2, size):
    """Split a partition-range copy (dst start s1, src start s2, length
    size) into engine-legal pieces."""
    out = []
    off = 0
    while off < size:
        take = min(size - off, part_cap((s1 + off) % 128),
                   part_cap((s2 + off) % 128))
        out.append((s1 + off, s2 + off, take))
        off += take
    return out


def head_fragments(o_lo, o_hi, base):
    """Split channel range [o_lo, o_hi) (global rows, head size 96 rel to
    `base`) into (head, d_lo, d_hi, p_lo, p_hi), p rel to o_lo."""
    frags = []
    g = o_lo
    while g < o_hi:
        h = (g - base) // D
        d_lo = (g - base) - h * D
        take = min(o_hi - g, D - d_lo)
        for (d0, p0, sz) in part_pieces2(d_lo, g - o_lo, take):
            frags.append((h, d0, d0 + sz, p0, p0 + sz))
        g += take
    return frags


def build(b_loc=B_LOC):
    import concourse.bass as bass  # noqa: F401
    import concourse.tile as tile
    import concourse.bacc as bacc
    from concourse import mybir

    from concourse.masks import make_identity

    f32 = mybir.dt.float32
    f32r = mybir.dt.float32r
    bf16 = mybir.dt.bfloat16
    AF = mybir.ActivationFunctionType
    OP = mybir.AluOpType

    assert b_loc % 2 == 0
    NG = b_loc // 2

    nc = bacc.Bacc("TRN2", target_bir_lowering=False)
    x_d = nc.dram_tensor("x", [b_loc, N, C], f32, kind="ExternalInput")
    qkvw_d = nc.dram_tensor("qkv_w", [3 * C, C], f32, kind="ExternalInput")
    projw_d = nc.dram_tensor("proj_w", [C, C], f32, kind="ExternalInput")
    projb_d = nc.dram_tensor("proj_b", [C], f32, kind="ExternalInput")
    out_d = nc.dram_tensor("out", [b_loc, N, C], f32, kind="ExternalOutput")

    from contextlib import ExitStack

    with tile.TileContext(nc) as tc, ExitStack() as es:
        if True:
            def P(**kw):
                return es.enter_context(tc.tile_pool(**kw))

            constp = P(name="const", bufs=1)
            wloadp = P(name="wload", bufs=3)
            xloadp = P(name="xload", bufs=2)
            xtgp = P(name="xtg", bufs=2)
            qkp = P(name="qk", bufs=2)
            vsbp = P(name="vsb", bufs=2)
            pep = P(name="pe", bufs=6)
            aop = P(name="ao", bufs=2)
            rbcp = P(name="rbc", bufs=3)
            osbp = P(name="osb", bufs=2)
            tinyp = P(name="tiny", bufs=2)
            psmm = P(name="ps_mm", bufs=4, space="PSUM")
            pss = P(name="ps_s", bufs=2, space="PSUM")
            psop = P(name="ps_o", bufs=2, space="PSUM")
            # ---------------- constants ----------------
            # Bias-extension master rows (contraction rows 96:128).
            # wmaster (q side): row0 w1 = -BIAS_RAW on img+aux queries,
            #   row1 w2 = -BIAS_RAW on lang+aux, row2 w3 = -BIAS_RAW on
            #   img+lang.
            # umaster (k side): row0 u1 = 1 on real-lang slots, row1 u2 =
            #   1 on lang-copy slots, row2 u3 = 1 on real-aux slots.
            wmaster = constp.tile([32, N], bf16)
            umaster = constp.tile([32, NK], bf16)
            nc.vector.memset(wmaster[:], 0.0)
            nc.vector.memset(umaster[:], 0.0)
            nc.vector.memset(wmaster[0:1, 0:S1], -BIAS_RAW)
            nc.vector.memset(wmaster[0:1, S2:N], -BIAS_RAW)
            nc.vector.memset(umaster[0:1, S1:S2], 1.0)
            # rows 1,2: build in [1, *] stages, DMA to the row (compute
            # engines cannot address partition 1/2; DMA can).
            w2row = constp.tile([1, N], bf16)
            nc.vector.memset(w2row[:], 0.0)
            nc.vector.memset(w2row[0:1, S1:N], -BIAS_RAW)
            w3row = constp.tile([1, N], bf16)
            nc.vector.memset(w3row[:], 0.0)
            nc.vector.memset(w3row[0:1, 0:S2], -BIAS_RAW)
            u2row = constp.tile([1, NK], bf16)
            nc.vector.memset(u2row[:], 0.0)
            nc.vector.memset(u2row[0:1, S2:N], 1.0)
            u3row = constp.tile([1, NK], bf16)
            nc.vector.memset(u3row[:], 0.0)
            nc.vector.memset(u3row[0:1, N:NK], 1.0)

            ident = constp.tile([128, 128], f32)
            make_identity(nc, ident[:])

            # ---------------- persistent weights ----------------
            # wTt[r][p, ci, col] = qkv_w[128r+col, 128ci+p]  (q/k rows)
            wTt = [constp.tile([128, NCH, 128], bf16, name=f"wTt{r}")
                   for r in range(12)]
            # vwT[p, ci, vc] = qkv_w[2C+vc, 128ci+p]
            vwT = constp.tile([128, NCH, C], bf16)
            # pwT[p, c, oc] = proj_w[oc, 128c+p]  (chunk-major)
            pwT = constp.tile([128, NCH, C], bf16)

            def prep_w_load(r0, nr):
                """Load row-chunks [r0, r0+nr) of the concatenated
                qkv_w|proj_w row space in one DMA (f32, [p, r, c])."""
                wl = wloadp.tile([128, 2, C], f32, tag="wl")
                if r0 < 18:
                    src = qkvw_d[r0 * 128:(r0 + nr) * 128, :]
                else:
                    src = projw_d[(r0 - 18) * 128:(r0 - 18 + nr) * 128, :]
                nc.sync.dma_start(
                    wl[:, 0:nr, :],
                    src.rearrange("(r p) c -> p r c", p=128))
                return wl

            def prep_w_chunk(wl, j, r):
                """PE-transpose row-chunk r (f32 load read as f32r, 1.5
                cyc/row) two 128-col chunks per PSUM tile; the psum drain
                does the f32->bf16 cast into the persistent home."""
                for cq in range(NCH // 2):
                    pt = psmm.tile([128, 2, 128], f32, tag="mm")
                    for q in range(2):
                        ci = 2 * cq + q
                        nc.tensor.transpose(
                            pt[:, q, :],
                            wl[:, j, ci * 128:(ci + 1) * 128],
                            ident[:])
                    if r < 12:
                        dst = wTt[r][:, 2 * cq:2 * cq + 2, :]
                    elif r < 18:
                        rr = r - 12
                        dst = vwT[:, 2 * cq:2 * cq + 2,
                                  rr * 128:(rr + 1) * 128]
                    else:
                        rr = r - 18
                        dst = pwT[:, 2 * cq:2 * cq + 2,
                                  rr * 128:(rr + 1) * 128]
                    if (r + cq) % 2 == 0:
                        nc.vector.tensor_copy(dst, pt[:])
                    else:
                        nc.scalar.copy(dst, pt[:])

            # proj bias row (bf16) + ones row for the K=1 bias matmul
            pbr = constp.tile([1, C], f32)
            nc.sync.dma_start(pbr[:], projb_d[None, :])
            pbb = constp.tile([1, C], bf16)
            nc.vector.tensor_copy(pbb[:], pbr[:])
            # pwTh[d, h, oc] = proj_w[oc, 96h+d]; row 96 (h=0) = proj_b
            pwTh = constp.tile([D + 1, H, C], bf16)

            # ---------------- per-group x load ----------------
            def x_dma(g):
                """Dispatch the two fused x DMAs for group g."""
                xfs = []
                for nt_i, (noff, nsz) in enumerate(NT):
                    xf = xloadp.tile([128, 2, C], f32, tag="xf")
                    nc.sync.dma_start(
                        xf[:nsz, :, :],
                        x_d[2 * g:2 * g + 2, noff:noff + nsz, :]
                        .rearrange("b p c -> p b c"))
                    xfs.append(xf)
                return xfs

            def x_transp(g, xfs):
                """PE-transpose loaded x (f32, 2 cyc/row) into the fused
                layout xTg [128c, ci, b, 256key]; psum drains cast to
                bf16.  Ext key slots 236:256 are the aux dup."""
                xtg = xtgp.tile([128, NCH, 2, NK], bf16, name="xTg")
                for nt_i, (noff, nsz) in enumerate(NT):
                    xf = xfs[nt_i]
                    for b_in in range(2):
                        for cq in range(NCH // 2):
                            pt = psmm.tile([128, 2, 128], f32, tag="mm")
                            for q in range(2):
                                ci = 2 * cq + q
                                nc.tensor.transpose(
                                    pt[:, q, 0:nsz],
                                    xf[:nsz, b_in,
                                       ci * 128:(ci + 1) * 128],
                                    ident[0:nsz, 0:nsz])
                            if (b_in + cq) % 2 == 0:
                                nc.vector.tensor_copy(
                                    xtg[:, 2 * cq:2 * cq + 2, b_in,
                                        noff:noff + nsz],
                                    pt[:, :, 0:nsz])
                            else:
                                nc.scalar.copy(
                                    xtg[:, 2 * cq:2 * cq + 2, b_in,
                                        noff:noff + nsz],
                                    pt[:, :, 0:nsz])
                # ext slots 236:256 <- dup of aux tokens 216:236
                nc.vector.tensor_copy(xtg[:, :, :, N:NK],
                                      xtg[:, :, :, S2:N])
                return {"xtg": xtg}

            def x_load(g):
                return x_transp(g, x_dma(g))

            # ---------------- per-group compute pieces ----------------
            def qk_chunk(oi, q_all, k_all, xtg, dr_i):
                ps = psmm.tile([128, 2, N], f32, tag="mm")
                for ci in range(NCH):
                    nc.tensor.matmul(
                        ps[:, :, :], wTt[oi][:, ci, :],
                        xtg[:, ci, :, 0:N],
                        start=(ci == 0), stop=(ci == NCH - 1),
                        skip_group_check=True)
                t = oi // 6
                for (h, d_lo, d_hi, p_lo, p_hi) in head_fragments(
                        oi * 128, (oi + 1) * 128, t * C):
                    if t == 0:
                        dst = q_all[d_lo:d_hi, h, :, :]
                    else:
                        dst = k_all[d_lo:d_hi, h, :, 0:N]
                    if dr_i[0] % 3 != 2:
                        nc.vector.tensor_copy(dst, ps[p_lo:p_hi, :, :])
                    else:
                        nc.scalar.copy(dst, ps[p_lo:p_hi, :, :])
                    dr_i[0] += 1

            def kext(k_all):
                # save real aux keys to 236:256, then overwrite 216:236
                # with the lang keys (pre-bias copy edit)
                nc.gpsimd.tensor_copy(k_all[0:D, :, :, N:NK],
                                      k_all[0:D, :, :, S2:N])
                nc.gpsimd.tensor_copy(k_all[0:D, :, :, S2:N],
                                      k_all[0:D, :, :, S1:S2])

            def v_half(g, b_in, jt, half, xtg, vp_t):
                if half == 0:
                    pv = psmm.tile([128, 5 * D], f32, tag="mm")
                    for ci in range(NCH):
                        nc.tensor.matmul(
                            pv[:, :],
                            xtg[:, ci, b_in, jt * 128:(jt + 1) * 128],
                            vwT[:, ci, 0:5 * D],
                            start=(ci == 0), stop=(ci == NCH - 1),
                            skip_group_check=True)
                    nc.scalar.copy(
                        vp_t[:, 0:5, 0:D],
                        pv[:].rearrange("p (h d) -> p h d", h=5))
                    if g < 2:
                        nc.vector.memset(vp_t[:, :, D:D + 1], 1.0)
                else:
                    pv = psmm.tile([128, 3 * D], f32, tag="mm")
                    for ci in range(NCH):
                        nc.tensor.matmul(
                            pv[:, :],
                            xtg[:, ci, b_in, jt * 128:(jt + 1) * 128],
                            vwT[:, ci, 5 * D:8 * D],
                            start=(ci == 0), stop=(ci == NCH - 1),
                            skip_group_check=True)
                    nc.vector.tensor_copy(
                        vp_t[:, 5:8, 0:D],
                        pv[:].rearrange("p (h d) -> p h d", h=3))

            def v_unit(g, b_in, jt, xtg, vp_t):
                v_half(g, b_in, jt, 0, xtg, vp_t)
                v_half(g, b_in, jt, 1, xtg, vp_t)

            def score_seg(b_in, hp, q_all, k_all):
                pes = []
                for jt in range(2):
                    psj = pss.tile([128, 2, N], f32, tag="s")
                    for hh in range(2):
                        h = 2 * hp + hh
                        nc.tensor.matmul(
                            psj[:, hh, :],
                            k_all[:, h, b_in, jt * 128:(jt + 1) * 128],
                            q_all[:, h, b_in, :],
                            start=True, stop=True, skip_group_check=True)
                    pe = pep.tile([128, 2, N], bf16, tag="pe")
                    nc.scalar.activation(pe[:], psj[:], AF.Exp, scale=SCALE)
                    pes.append(pe)
                return pes

            # normalize-multiply fragments for the chunk-major aoT
            # layout: head h channels 96h..96h+96 land at (chunk c = ch
            # //128, partition ch%128), split at engine-legal starts.
            AO_FRAGS = [[] for _ in range(H // 2)]
            for h in range(H):
                ch = D * h
                while ch < D * (h + 1):
                    c, p = ch // 128, ch % 128
                    take = min(D * (h + 1) - ch, 128 - p)
                    for (p0, d0, sz) in part_pieces2(p, ch - D * h, take):
                        AO_FRAGS[h // 2].append(
                            (c, p0, p0 + sz, h % 2, d0, d0 + sz))
                    ch += take

            def attnv_seg(b_in, hp, pes, vps, aoT):
                pso = psop.tile([D + 1, 2, N], f32, tag="o")
                for hh in range(2):
                    h = 2 * hp + hh
                    for jt in range(2):
                        nc.tensor.matmul(
                            pso[:, hh, :], vps[b_in][jt][:, h, :],
                            pes[jt][:, hh, :],
                            start=(jt == 0), stop=(jt == 1),
                            skip_group_check=True)
                den = tinyp.tile([1, 2, N], f32, tag="den")
                nc.vector.tensor_copy(den[:], pso[D:D + 1, :, :])
                rf = tinyp.tile([1, 2, N], f32, tag="rf")
                nc.vector.reciprocal_approx_fast(rf[:], den[:])
                rbc = rbcp.tile([128, 2, N], f32, tag="rbc")
                nc.gpsimd.partition_broadcast(
                    rbc[:], rf[0:1, :, :].rearrange("p a b -> p (a b)"))
                nc.vector.tensor_tensor(
                    aoT[0:D, 2 * hp:2 * hp + 2, b_in, :], pso[0:D, :, :],
                    rbc[0:D, :, :], OP.mult)

            def proj_fg(b, b_in, nt_i, fg, aoT, osb_t):
                noff, nsz = NT[nt_i]
                f0, fsz = (0, 512) if fg == 0 else (512, 256)
                pp = psmm.tile([128, 512], f32, tag="mm")
                for h in range(H):
                    kk = D + 1 if h == 0 else D
                    nc.tensor.matmul(
                        pp[:nsz, :fsz],
                        aoT[0:kk, h, b_in, noff:noff + nsz],
                        pwTh[0:kk, h, f0:f0 + fsz],
                        start=(h == 0), stop=(h == H - 1),
                        skip_group_check=True)
                nc.vector.tensor_copy(osb_t[:nsz, f0:f0 + fsz],
                                      pp[:nsz, :fsz])
                if fg == 1:
                    nc.scalar.dma_start(out_d[b, noff:noff + nsz, :],
                                        osb_t[:nsz])

            def new_group_tiles(g):
                q_all = qkp.tile([KEXT, H, 2, N], bf16, name="q_all")
                k_all = qkp.tile([KEXT, H, 2, NK], bf16, name="k_all")
                aoT = aop.tile([D + 1, H, 2, N], bf16, name="aoT")
                vps = [[vsbp.tile([128, H, D + 1], bf16,
                                  name=f"vp{b_in}_{jt}")
                        for jt in range(2)] for b_in in range(2)]
                if g < 2:
                    nc.vector.memset(aoT[D:D + 1, 0, :, :], 1.0)
                return q_all, k_all, aoT, vps

            def seed_bias_rows(q_all, k_all):
                # bias-extension rows (needed only by NEXT group's score
                # matmuls; issued after this group's segments so the
                # gpsimd queue serves the normalize broadcasts first)
                nc.gpsimd.tensor_copy(
                    q_all[96:128, :, :, :],
                    wmaster[:, None, None, :].to_broadcast(
                        (32, H, 2, N)))
                nc.gpsimd.tensor_copy(
                    k_all[96:128, :, :, :],
                    umaster[:, None, None, :].to_broadcast(
                        (32, H, 2, NK)))

            def proj_b_in(prev, b_in):
                b = 2 * prev["g"] + b_in
                osb_t = osbp.tile([128, C], f32, tag="osb")
                for nt_i in range(2):
                    for fg in range(2):
                        proj_fg(b, b_in, nt_i, fg, prev["aoT"], osb_t)
                    osb_t = osb_t if nt_i == 1 else osbp.tile(
                        [128, C], f32, tag="osb")

            # ---------------- schedule ----------------
            # All load-DMAs dispatch on SP ahead of the transpose chain
            # (which waits on casts) so the transfers stream
            # back-to-back; transposes dispatch on the Activation hwdge
            # queue as their casts complete.
            xcur = x_load(0)
            # load plan: tiny first load so wTt[0] lands early, then
            # 2-chunk loads, 3-deep pipeline; x(1) prefetch goes ahead
            # of the proj weights (not needed until group-1's tail)
            LDS = [(0, 1), (1, 2), (3, 2), (5, 2), (7, 2), (9, 2),
                   (11, 1), (12, 2), (14, 2), (16, 2), (18, 2),
                   (20, 2), (22, 2)]
            wb_q = [prep_w_load(*LDS[0]), prep_w_load(*LDS[1]),
                    prep_w_load(*LDS[2])]
            nc.sync.dma_start(wmaster[1:2, :], w2row[:])
            nc.sync.dma_start(wmaster[2:3, :], w3row[:])
            nc.sync.dma_start(umaster[1:2, :], u2row[:])
            nc.sync.dma_start(umaster[2:3, :], u3row[:])
            xnext = None
            g0_tiles = new_group_tiles(0)
            g0_dr = [0]
            for li in range(len(LDS)):
                wlt = wb_q.pop(0)
                r0, nr = LDS[li]
                for j in range(nr):
                    r = r0 + j
                    prep_w_chunk(wlt, j, r)
                    if r < 12:
                        # group-0 q/k chunk right behind its weight tile
                        qk_chunk(r, g0_tiles[0], g0_tiles[1],
                                 xcur["xtg"], g0_dr)
                nli = li + 3
                if nli < len(LDS):
                    wb_q.append(prep_w_load(*LDS[nli]))
                if li == 9 and NG > 1:
                    xnext = x_load(1)
            # one-time reshuffle chunk-major pwT -> per-head pwTh, in
            # the prologue's DMA-bound tail where DVE/ACT are idle
            ei = 0
            for h in range(H):
                ch = D * h
                while ch < D * (h + 1):
                    c, p = ch // 128, ch % 128
                    take = min(D * (h + 1) - ch, 128 - p)
                    for (p0, d0, sz) in part_pieces2(p, ch - D * h, take):
                        if ei % 2 == 0:
                            nc.vector.tensor_copy(
                                pwTh[d0:d0 + sz, h, :],
                                pwT[p0:p0 + sz, c, :])
                        else:
                            nc.scalar.copy(
                                pwTh[d0:d0 + sz, h, :],
                                pwT[p0:p0 + sz, c, :])
                        ei += 1
                    ch += take
            nc.vector.tensor_copy(pwTh[D:D + 1, 0, :], pbb[:])

            prev = None
            for g in range(NG):
                if g == 0:
                    q_all, k_all, aoT, vps = g0_tiles
                    dr_i = g0_dr
                else:
                    q_all, k_all, aoT, vps = new_group_tiles(g)
                    dr_i = [0]
                osb_p0 = [None, None]
                if prev is None:
                    pass
                else:
                    bp0 = 2 * prev["g"]
                    osb_pa = osbp.tile([128, C], f32, tag="osb")
                    osb_pb = osbp.tile([128, C], f32, tag="osb")
                    osb_p0 = [osb_pa, osb_pb]
                    for oi in range(4):
                        qk_chunk(oi, q_all, k_all, xcur["xtg"], dr_i)
                    oi = 4
                    # v half-units for THIS group's b0 and prev-b0 proj
                    # units ride the segments as extra exp-hiding fillers
                    vh = [(0, 0, 0), (0, 0, 1), (0, 1, 0), (0, 1, 1)]
                    for b_in in range(2):
                        for hp in range(H // 2):
                            pes = score_seg(b_in, hp, prev["q_all"],
                                            prev["k_all"])
                            qk_chunk(oi, q_all, k_all, xcur["xtg"], dr_i)
                            oi += 1
                            si = 4 * b_in + hp
                            if 1 <= si <= 4:
                                vb, vj, vhf = vh[si - 1]
                                v_half(g, vb, vj, vhf, xcur["xtg"],
                                       vps[vb][vj])
                            elif si >= 5:
                                nt_i, fg = [(0, 0), (0, 1),
                                            (1, 0)][si - 5]
                                proj_fg(bp0, 0, nt_i, fg, prev["aoT"],
                                        osb_p0[nt_i])
                            attnv_seg(b_in, hp, pes, prev["vps"],
                                      prev["aoT"])
                xfs_next = x_dma(g + 2) if g + 2 < NG else None
                if g < 2:
                    seed_bias_rows(q_all, k_all)
                kext(k_all)
                # tail: remaining v units + remaining proj for g-1
                if prev is None:
                    for b_in in range(2):
                        for jt in range(2):
                            v_unit(g, b_in, jt, xcur["xtg"],
                                   vps[b_in][jt])
                elif g < NG - 1:
                    # b0's v went into the segments; b1's v here
                    v_unit(g, 1, 0, xcur["xtg"], vps[1][0])
                    proj_fg(2 * prev["g"], 0, 1, 1, prev["aoT"],
                            osb_p0[1])
                    v_unit(g, 1, 1, xcur["xtg"], vps[1][1])
                    proj_b_in(prev, 1)
                else:
                    # LAST group: pull its own b0 attention into the
                    # tail, with the remaining tail work as fillers, so
                    # the epilogue only handles b1.
                    v_half(g, 1, 0, 0, xcur["xtg"], vps[1][0])
                    tail_f = [
                        lambda: v_half(g, 1, 0, 1, xcur["xtg"],
                                       vps[1][0]),
                        lambda: proj_fg(2 * prev["g"], 0, 1, 1,
                                        prev["aoT"], osb_p0[1]),
                        lambda: v_half(g, 1, 1, 0, xcur["xtg"],
                                       vps[1][1]),
                        lambda: v_half(g, 1, 1, 1, xcur["xtg"],
                                       vps[1][1]),
                    ]
                    for hp in range(H // 2):
                        pes = score_seg(0, hp, q_all, k_all)
                        tail_f[hp]()
                        attnv_seg(0, hp, pes, vps, aoT)
                    proj_b_in(prev, 1)
                prev = {"g": g, "q_all": q_all, "k_all": k_all,
                        "aoT": aoT, "vps": vps}
                xcur = xnext
                # transposes issue here, AFTER their DMA (dispatched
                # mid-group) has landed - no PE queue-head stall
                xnext = (x_transp(g + 2, xfs_next)
                         if g + 2 < NG else None)

            # ---------------- epilogue: last group's b1 ---------------
            # (b0's attention ran in the last group's tail)
            # b1 segments with proj(b0) units as fillers
            bL = 2 * prev["g"]
            osb0 = osbp.tile([128, C], f32, tag="osb")
            osb1 = osbp.tile([128, C], f32, tag="osb")
            fillers = [(0, 0, osb0), (0, 1, osb0), (1, 0, osb1),
                       (1, 1, osb1)]
            for hp in range(H // 2):
                pes = score_seg(1, hp, prev["q_all"], prev["k_all"])
                nt_i, fg, ot = fillers[hp]
                proj_fg(bL, 0, nt_i, fg, prev["aoT"], ot)
                attnv_seg(1, hp, pes, prev["vps"], prev["aoT"])
            proj_b_in(prev, 1)

    nc.compile()
    return nc


_NC_CACHE = {}


def _get_nc(b_loc):
    if b_loc not in _NC_CACHE:
        _NC_CACHE[b_loc] = build(b_loc)
    return _NC_CACHE[b_loc]


def _run(inputs, trace=False):
    from concourse.bass_utils import run_bass_kernel_spmd

    x = np.ascontiguousarray(np.asarray(inputs["x"], dtype=np.float32))
    qkv_w = np.ascontiguousarray(np.asarray(inputs["qkv_w"], dtype=np.float32))
    proj_w = np.ascontiguousarray(np.asarray(inputs["proj_w"], dtype=np.float32))
    proj_b = np.ascontiguousarray(np.asarray(inputs["proj_b"], dtype=np.float32))

    nc = _get_nc(B_LOC)
    in_maps = [
        {
            "x": np.ascontiguousarray(x[i * B_LOC:(i + 1) * B_LOC]),
            "qkv_w": qkv_w,
            "proj_w": proj_w,
            "proj_b": proj_b,
        }
        for i in range(N_CORES)
    ]
    res = run_bass_kernel_spmd(
        nc, in_maps, core_ids=list(range(N_CORES)), trace=trace)
    out = np.concatenate([r["out"] for r in res.results], axis=0)
    return out, res


def kernel(x, qkv_w, proj_w, proj_b):
    out, _ = _run({"x": x, "qkv_w": qkv_w, "proj_w": proj_w,
                   "proj_b": proj_b})
    return out


# revision 68
# speedup vs baseline: 1.0224x; 1.0224x over previous
"""Trainium2 Bass kernel for the sparse_attention PoC block.

Reference (per batch item):
  qkv = x @ qkv_w.T; q,k,v heads [H, N, D]
  attn = (q @ k.T) * scale; block edits: attn[:S1, S2:] = attn[:S1, S1:S2]
  (pre-bias copy), -100 bias on [:S1, S1:S2], [S1:S2, S2:], [S2:, S1:S2];
  softmax; attn @ v; proj + bias.

Distribution: data-parallel over batch B=64 across 8 cores (8 items per
core, weights replicated, no collectives).

Per-core design (all matmuls bf16, fp32 PSUM):
  - x and the weights are transposed on the PE (f32 in, 2 cyc/row,
    identity matmul); the PSUM drain does the f32->bf16 cast, so there
    are no standalone cast instructions.  (DMA XBAR transposes were
    tried and are FASTER per op, but racy: concurrent dma_start_transpose
    dispatch from two queues corrupts data nondeterministically.)
  - Batches processed in groups of 2; q/k matmuls fuse both batches into
    one rhs [128c, 2, 236] (472-column streams).
  - k uses a 256-wide EXTENDED key axis: 0:216 real tokens, 216:236 lang
    keys (the pre-bias "copy" edit), 236:256 real aux keys.  Rank-3 bias
    rows (u1/u2/u3 on the k side x w1/w2/w3 on the q side, contraction
    rows 96:99) route each query group to the right slots.
  - scores sT[j, n] = k_ext.T @ q_ext per (head, key-tile); softmax
    without max-subtraction (suppressed entries underflow exp to 0);
    exp on ScalarE with 1/sqrt(D) folded in.
  - v packed [128j, H, D+1] ([v | ones]); attn@v gives oT [D+1, n] with
    a denominator row; normalize via fast reciprocal + gpsimd
    partition_broadcast; proj per head (K=96, +bias row on head 0).
  - SOFTWARE PIPELINE: group g's qk-chunk matmuls are issued BETWEEN the
    score and attn@v matmuls of group g-1's attention so the PE never
    stalls waiting on ScalarE's exp (PE p-state ramp: idle gaps drop the
    clock 2.4GHz -> 1.2GHz).  v/proj/x-prep fill the group tail; group
    0's qk chunks interleave with the weight-prep transposes so the PE
    starts ~13us into the kernel.

Engine split: PE matmuls+transposes; DVE/ScalarE split the psum drains
(ScalarE also exp); Pool(gpsimd) sbuf->sbuf dup/bias-rows/broadcast (no
PSUM port - and it is ~3x slower per element than DVE, so only
off-critical-path work goes there); SP = input DMA only, out-DMAs ride
the Activation hwdge queue behind their drains.

Partition rule (walrus): compute-engine APs start at partition 0/32/64/96
(max 128/32/64/32); matmul operands start at partition 0.
"""

import numpy as np

B, N, C = 64, 236, 768
H, D = 8, 96
S1, S2 = 196, 216
BIAS = 100.0
SCALE = D ** -0.5
BIAS_RAW = BIAS / SCALE  # applied on raw (pre-scale) scores

N_CORES = 8
B_LOC = B // N_CORES

NT = [(0, 128), (128, 108)]  # token tiles
NCH = C // 128  # 6 contraction chunks over C
KEXT = 128  # score contraction: 96 head dims + bias rows
NK = 256  # extended key axis


def part_cap(s):
    return 128 if s == 0 else 64 if s == 64 else 32


def part_pieces2(s1, s2, size):
    """Split a partition-range copy (dst start s1, src start s2, length
    size) into engine-legal pieces."""
    out = []
    off = 0
    while off < size:
        take = min(size - off, part_cap((s1 + off) % 128),
                   part_cap((s2 + off) % 128))
        out.append((s1 + off, s2 + off, take))
        off += take
    return out


def head_fragments(o_lo, o_hi, base):
    """Split channel range [o_lo, o_hi) (global rows, head size 96 rel to
    `base`) into (head, d_lo, d_hi, p_lo, p_hi), p rel to o_lo."""
    frags = []
    g = o_lo
    while g < o_hi:
        h = (g - base) // D
        d_lo = (g - base) - h * D
        take = min(o_hi - g, D - d_lo)
        for (d0, p0, sz) in part_pieces2(d_lo, g - o_lo, take):
            frags.append((h, d0, d0 + sz, p0, p0 + sz))
        g += take
    return frags


def build(b_loc=B_LOC):
    import concourse.bass as bass  # noqa: F401
    import concourse.tile as tile
    import concourse.bacc as bacc
    from concourse import mybir

    from concourse.masks import make_identity

    f32 = mybir.dt.float32
    f32r = mybir.dt.float32r
    bf16 = mybir.dt.bfloat16
    AF = mybir.ActivationFunctionType
    OP = mybir.AluOpType

    assert b_loc % 2 == 0
    NG = b_loc // 2

    nc = bacc.Bacc("TRN2", target_bir_lowering=False)
    x_d = nc.dram_tensor("x", [b_loc, N, C], f32, kind="ExternalInput")
    qkvw_d = nc.dram_tensor("qkv_w", [3 * C, C], f32, kind="ExternalInput")
    projw_d = nc.dram_tensor("proj_w", [C, C], f32, kind="ExternalInput")
    projb_d = nc.dram_tensor("proj_b", [C], f32, kind="ExternalInput")
    out_d = nc.dram_tensor("out", [b_loc, N, C], f32, kind="ExternalOutput")

    from contextlib import ExitStack

    with tile.TileContext(nc) as tc, ExitStack() as es:
        if True:
            def P(**kw):
                return es.enter_context(tc.tile_pool(**kw))

            constp = P(name="const", bufs=1)
            wloadp = P(name="wload", bufs=3)
            xloadp = P(name="xload", bufs=2)
            xtgp = P(name="xtg", bufs=2)
            qkp = P(name="qk", bufs=2)
            vsbp = P(name="vsb", bufs=2)
            pep = P(name="pe", bufs=6)
            aop = P(name="ao", bufs=2)
            rbcp = P(name="rbc", bufs=3)
            osbp = P(name="osb", bufs=2)
            tinyp = P(name="tiny", bufs=2)
            psmm = P(name="ps_mm", bufs=4, space="PSUM")
            pss = P(name="ps_s", bufs=2, space="PSUM")
            psop = P(name="ps_o", bufs=2, space="PSUM")
            # ---------------- constants ----------------
            # Bias-extension master rows (contraction rows 96:128).
            # wmaster (q side): row0 w1 = -BIAS_RAW on img+aux queries,
            #   row1 w2 = -BIAS_RAW on lang+aux, row2 w3 = -BIAS_RAW on
            #   img+lang.
            # umaster (k side): row0 u1 = 1 on real-lang slots, row1 u2 =
            #   1 on lang-copy slots, row2 u3 = 1 on real-aux slots.
            wmaster = constp.tile([32, N], bf16)
            umaster = constp.tile([32, NK], bf16)
            nc.vector.memset(wmaster[:], 0.0)
            nc.vector.memset(umaster[:], 0.0)
            nc.vector.memset(wmaster[0:1, 0:S1], -BIAS_RAW)
            nc.vector.memset(wmaster[0:1, S2:N], -BIAS_RAW)
            nc.vector.memset(umaster[0:1, S1:S2], 1.0)
            # rows 1,2: build in [1, *] stages, DMA to the row (compute
            # engines cannot address partition 1/2; DMA can).
            w2row = constp.tile([1, N], bf16)
            nc.vector.memset(w2row[:], 0.0)
            nc.vector.memset(w2row[0:1, S1:N], -BIAS_RAW)
            w3row = constp.tile([1, N], bf16)
            nc.vector.memset(w3row[:], 0.0)
            nc.vector.memset(w3row[0:1, 0:S2], -BIAS_RAW)
            u2row = constp.tile([1, NK], bf16)
            nc.vector.memset(u2row[:], 0.0)
            nc.vector.memset(u2row[0:1, S2:N], 1.0)
            u3row = constp.tile([1, NK], bf16)
            nc.vector.memset(u3row[:], 0.0)
            nc.vector.memset(u3row[0:1, N:NK], 1.0)

            ident = constp.tile([128, 128], f32)
            make_identity(nc, ident[:])

            # ---------------- persistent weights ----------------
            # wTt[r][p, ci, col] = qkv_w[128r+col, 128ci+p]  (q/k rows)
            wTt = [constp.tile([128, NCH, 128], bf16, name=f"wTt{r}")
                   for r in range(12)]
            # vwT[p, ci, vc] = qkv_w[2C+vc, 128ci+p]
            vwT = constp.tile([128, NCH, C], bf16)
            # pwT[p, c, oc] = proj_w[oc, 128c+p]  (chunk-major)
            pwT = constp.tile([128, NCH, C], bf16)

            def prep_w_load(r0, nr):
                """Load row-chunks [r0, r0+nr) of the concatenated
                qkv_w|proj_w row space in one DMA (f32, [p, r, c])."""
                wl = wloadp.tile([128, 2, C], f32, tag="wl")
                if r0 < 18:
                    src = qkvw_d[r0 * 128:(r0 + nr) * 128, :]
                else:
                    src = projw_d[(r0 - 18) * 128:(r0 - 18 + nr) * 128, :]
                nc.sync.dma_start(
                    wl[:, 0:nr, :],
                    src.rearrange("(r p) c -> p r c", p=128))
                return wl

            def prep_w_chunk(wl, j, r):
                """PE-transpose row-chunk r (f32 load read as f32r, 1.5
                cyc/row) two 128-col chunks per PSUM tile; the psum drain
                does the f32->bf16 cast into the persistent home."""
                for cq in range(NCH // 2):
                    pt = psmm.tile([128, 2, 128], f32, tag="mm")
                    for q in range(2):
                        ci = 2 * cq + q
                        nc.tensor.transpose(
                            pt[:, q, :],
                            wl[:, j, ci * 128:(ci + 1) * 128],
                            ident[:])
                    if r < 12:
                        dst = wTt[r][:, 2 * cq:2 * cq + 2, :]
                    elif r < 18:
                        rr = r - 12
                        dst = vwT[:, 2 * cq:2 * cq + 2,
                                  rr * 128:(rr + 1) * 128]
                    else:
                        rr = r - 18
                        dst = pwT[:, 2 * cq:2 * cq + 2,
                                  rr * 128:(rr + 1) * 128]
                    if (r + cq) % 2 == 0:
                        nc.vector.tensor_copy(dst, pt[:])
                    else:
                        nc.scalar.copy(dst, pt[:])

            # proj bias row (bf16) + ones row for the K=1 bias matmul
            pbr = constp.tile([1, C], f32)
            nc.sync.dma_start(pbr[:], projb_d[None, :])
            pbb = constp.tile([1, C], bf16)
            nc.vector.tensor_copy(pbb[:], pbr[:])
            # pwTh[d, h, oc] = proj_w[oc, 96h+d]; row 96 (h=0) = proj_b
            pwTh = constp.tile([D + 1, H, C], bf16)

            # ---------------- per-group x load ----------------
            def x_dma(g):
                """Dispatch the two fused x DMAs for group g."""
                xfs = []
                for nt_i, (noff, nsz) in enumerate(NT):
                    xf = xloadp.tile([128, 2, C], f32, tag="xf")
                    nc.sync.dma_start(
                        xf[:nsz, :, :],
                        x_d[2 * g:2 * g + 2, noff:noff + nsz, :]
                        .rearrange("b p c -> p b c"))
                    xfs.append(xf)
                return xfs

            def x_transp(g, xfs):
                """PE-transpose loaded x (f32, 2 cyc/row) into the fused
                layout xTg [128c, ci, b, 256key]; psum drains cast to
                bf16.  Ext key slots 236:256 are the aux dup."""
                xtg = xtgp.tile([128, NCH, 2, NK], bf16, name="xTg")
                for nt_i, (noff, nsz) in enumerate(NT):
                    xf = xfs[nt_i]
                    for b_in in range(2):
                        for cq in range(NCH // 2):
                            pt = psmm.tile([128, 2, 128], f32, tag="mm")
                            for q in range(2):
                                ci = 2 * cq + q
                                nc.tensor.transpose(
                                    pt[:, q, 0:nsz],
                                    xf[:nsz, b_in,
                                       ci * 128:(ci + 1) * 128],
                                    ident[0:nsz, 0:nsz])
                            if (b_in + cq) % 2 == 0:
                                nc.vector.tensor_copy(
                                    xtg[:, 2 * cq:2 * cq + 2, b_in,
                                        noff:noff + nsz],
                                    pt[:, :, 0:nsz])
                            else:
                                nc.scalar.copy(
                                    xtg[:, 2 * cq:2 * cq + 2, b_in,
                                        noff:noff + nsz],
                                    pt[:, :, 0:nsz])
                # ext slots 236:256 <- dup of aux tokens 216:236
                nc.vector.tensor_copy(xtg[:, :, :, N:NK],
                                      xtg[:, :, :, S2:N])
                return {"xtg": xtg}

            def x_load(g):
                return x_transp(g, x_dma(g))

            # ---------------- per-group compute pieces ----------------
            def qk_chunk(oi, q_all, k_all, xtg, dr_i):
                ps = psmm.tile([128, 2, N], f32, tag="mm")
                for ci in range(NCH):
                    nc.tensor.matmul(
                        ps[:, :, :], wTt[oi][:, ci, :],
                        xtg[:, ci, :, 0:N],
                        start=(ci == 0), stop=(ci == NCH - 1),
                        skip_group_check=True)
                t = oi // 6
                for (h, d_lo, d_hi, p_lo, p_hi) in head_fragments(
                        oi * 128, (oi + 1) * 128, t * C):
                    if t == 0:
                        dst = q_all[d_lo:d_hi, h, :, :]
                    else:
                        dst = k_all[d_lo:d_hi, h, :, 0:N]
                    if dr_i[0] % 3 != 2:
                        nc.vector.tensor_copy(dst, ps[p_lo:p_hi, :, :])
                    else:
                        nc.scalar.copy(dst, ps[p_lo:p_hi, :, :])
                    dr_i[0] += 1

            def kext(k_all):
                # save real aux keys to 236:256, then overwrite 216:236
                # with the lang keys (pre-bias copy edit)
                nc.gpsimd.tensor_copy(k_all[0:D, :, :, N:NK],
                                      k_all[0:D, :, :, S2:N])
                nc.gpsimd.tensor_copy(k_all[0:D, :, :, S2:N],
                                      k_all[0:D, :, :, S1:S2])

            def v_half(g, b_in, jt, half, xtg, vp_t):
                if half == 0:
                    pv = psmm.tile([128, 5 * D], f32, tag="mm")
                    for ci in range(NCH):
                        nc.tensor.matmul(
                            pv[:, :],
                            xtg[:, ci, b_in, jt * 128:(jt + 1) * 128],
                            vwT[:, ci, 0:5 * D],
                            start=(ci == 0), stop=(ci == NCH - 1),
                            skip_group_check=True)
                    nc.scalar.copy(
                        vp_t[:, 0:5, 0:D],
                        pv[:].rearrange("p (h d) -> p h d", h=5))
                    if g < 2:
                        nc.vector.memset(vp_t[:, :, D:D + 1], 1.0)
                else:
                    pv = psmm.tile([128, 3 * D], f32, tag="mm")
                    for ci in range(NCH):
                        nc.tensor.matmul(
                            pv[:, :],
                            xtg[:, ci, b_in, jt * 128:(jt + 1) * 128],
                            vwT[:, ci, 5 * D:8 * D],
                            start=(ci == 0), stop=(ci == NCH - 1),
                            skip_group_check=True)
                    nc.vector.tensor_copy(
                        vp_t[:, 5:8, 0:D],
                        pv[:].rearrange("p (h d) -> p h d", h=3))

            def v_unit(g, b_in, jt, xtg, vp_t):
                v_half(g, b_in, jt, 0, xtg, vp_t)
                v_half(g, b_in, jt, 1, xtg, vp_t)

            def score_seg(b_in, hp, q_all, k_all):
                pes = []
                for jt in range(2):
                    psj = pss.tile([128, 2, N], f32, tag="s")
                    for hh in range(2):
                        h = 2 * hp + hh
                        nc.tensor.matmul(
                            psj[:, hh, :],
                            k_all[:, h, b_in, jt * 128:(jt + 1) * 128],
                            q_all[:, h, b_in, :],
                            start=True, stop=True, skip_group_check=True)
                    pe = pep.tile([128, 2, N], bf16, tag="pe")
                    nc.scalar.activation(pe[:], psj[:], AF.Exp, scale=SCALE)
                    pes.append(pe)
                return pes

            # normalize-multiply fragments for the chunk-major aoT
            # layout: head h channels 96h..96h+96 land at (chunk c = ch
            # //128, partition ch%128), split at engine-legal starts.
            AO_FRAGS = [[] for _ in range(H // 2)]
            for h in range(H):
                ch = D * h
                while ch < D * (h + 1):
                    c, p = ch // 128, ch % 128
                    take = min(D * (h + 1) - ch, 128 - p)
                    for (p0, d0, sz) in part_pieces2(p, ch - D * h, take):
                        AO_FRAGS[h // 2].append(
                            (c, p0, p0 + sz, h % 2, d0, d0 + sz))
                    ch += take

            def attnv_seg(b_in, hp, pes, vps, aoT):
                pso = psop.tile([D + 1, 2, N], f32, tag="o")
                for hh in range(2):
                    h = 2 * hp + hh
                    for jt in range(2):
                        nc.tensor.matmul(
                            pso[:, hh, :], vps[b_in][jt][:, h, :],
                            pes[jt][:, hh, :],
                            start=(jt == 0), stop=(jt == 1),
                            skip_group_check=True)
                den = tinyp.tile([1, 2, N], f32, tag="den")
                nc.scalar.copy(den[:], pso[D:D + 1, :, :])
                rf = tinyp.tile([1, 2, N], f32, tag="rf")
                nc.vector.reciprocal_approx_fast(rf[:], den[:])
                rbc = rbcp.tile([128, 2, N], f32, tag="rbc")
                nc.gpsimd.partition_broadcast(
                    rbc[:], rf[0:1, :, :].rearrange("p a b -> p (a b)"))
                nc.vector.tensor_tensor(
                    aoT[0:D, 2 * hp:2 * hp + 2, b_in, :], pso[0:D, :, :],
                    rbc[0:D, :, :], OP.mult)

            def proj_fg(b, b_in, nt_i, fg, aoT, osb_t):
                noff, nsz = NT[nt_i]
                f0, fsz = (0, 512) if fg == 0 else (512, 256)
                pp = psmm.tile([128, 512], f32, tag="mm")
                for h in range(H):
                    kk = D + 1 if h == 0 else D
                    nc.tensor.matmul(
                        pp[:nsz, :fsz],
                        aoT[0:kk, h, b_in, noff:noff + nsz],
                        pwTh[0:kk, h, f0:f0 + fsz],
                        start=(h == 0), stop=(h == H - 1),
                        skip_group_check=True)
                if fg == 0:
                    nc.scalar.copy(osb_t[:nsz, f0:f0 + fsz],
                                   pp[:nsz, :fsz])
                else:
                    nc.vector.tensor_copy(osb_t[:nsz, f0:f0 + fsz],
                                          pp[:nsz, :fsz])
                    nc.scalar.dma_start(out_d[b, noff:noff + nsz, :],
                                        osb_t[:nsz])

            def new_group_tiles(g):
                q_all = qkp.tile([KEXT, H, 2, N], bf16, name="q_all")
                k_all = qkp.tile([KEXT, H, 2, NK], bf16, name="k_all")
                aoT = aop.tile([D + 1, H, 2, N], bf16, name="aoT")
                vps = [[vsbp.tile([128, H, D + 1], bf16,
                                  name=f"vp{b_in}_{jt}")
                        for jt in range(2)] for b_in in range(2)]
                if g < 2:
                    nc.vector.memset(aoT[D:D + 1, 0, :, :], 1.0)
                return q_all, k_all, aoT, vps

            def seed_bias_rows(q_all, k_all):
                # bias-extension rows (needed only by NEXT group's score
                # matmuls; issued after this group's segments so the
                # gpsimd queue serves the normalize broadcasts first)
                nc.gpsimd.tensor_copy(
                    q_all[96:128, :, :, :],
                    wmaster[:, None, None, :].to_broadcast(
                        (32, H, 2, N)))
                nc.gpsimd.tensor_copy(
                    k_all[96:128, :, :, :],
                    umaster[:, None, None, :].to_broadcast(
                        (32, H, 2, NK)))

            def proj_b_in(prev, b_in):
                b = 2 * prev["g"] + b_in
                osb_t = osbp.tile([128, C], f32, tag="osb")
                for nt_i in range(2):
                    for fg in range(2):
                        proj_fg(b, b_in, nt_i, fg, prev["aoT"], osb_t)
                    osb_t = osb_t if nt_i == 1 else osbp.tile(
                        [128, C], f32, tag="osb")

            # ---------------- schedule ----------------
            # All load-DMAs dispatch on SP ahead of the transpose chain
            # (which waits on casts) so the transfers stream
            # back-to-back; transposes dispatch on the Activation hwdge
            # queue as their casts complete.
            xcur = x_load(0)
            # load plan: tiny first load so wTt[0] lands early, then
            # 2-chunk loads, 3-deep pipeline; x(1) prefetch goes ahead
            # of the proj weights (not needed until group-1's tail)
            LDS = [(0, 1), (1, 2), (3, 2), (5, 2), (7, 2), (9, 2),
                   (11, 1), (12, 2), (14, 2), (16, 2), (18, 2),
                   (20, 2), (22, 2)]
            wb_q = [prep_w_load(*LDS[0]), prep_w_load(*LDS[1]),
                    prep_w_load(*LDS[2])]
            nc.sync.dma_start(wmaster[1:2, :], w2row[:])
            nc.sync.dma_start(wmaster[2:3, :], w3row[:])
            nc.sync.dma_start(umaster[1:2, :], u2row[:])
            nc.sync.dma_start(umaster[2:3, :], u3row[:])
            xnext = None
            g0_tiles = new_group_tiles(0)
            g0_dr = [0]
            for li in range(len(LDS)):
                wlt = wb_q.pop(0)
                r0, nr = LDS[li]
                for j in range(nr):
                    r = r0 + j
                    prep_w_chunk(wlt, j, r)
                    if r < 12:
                        # group-0 q/k chunk right behind its weight tile
                        qk_chunk(r, g0_tiles[0], g0_tiles[1],
                                 xcur["xtg"], g0_dr)
                nli = li + 3
                if nli < len(LDS):
                    wb_q.append(prep_w_load(*LDS[nli]))
                if li == 9 and NG > 1:
                    xnext = x_load(1)
            # one-time reshuffle chunk-major pwT -> per-head pwTh, in
            # the prologue's DMA-bound tail where DVE/ACT are idle
            ei = 0
            for h in range(H):
                ch = D * h
                while ch < D * (h + 1):
                    c, p = ch // 128, ch % 128
                    take = min(D * (h + 1) - ch, 128 - p)
                    for (p0, d0, sz) in part_pieces2(p, ch - D * h, take):
                        if ei % 2 == 0:
                            nc.vector.tensor_copy(
                                pwTh[d0:d0 + sz, h, :],
                                pwT[p0:p0 + sz, c, :])
                        else:
                            nc.scalar.copy(
                                pwTh[d0:d0 + sz, h, :],
                                pwT[p0:p0 + sz, c, :])
                        ei += 1
                    ch += take
            nc.vector.tensor_copy(pwTh[D:D + 1, 0, :], pbb[:])

            prev = None
            for g in range(NG):
                if g == 0:
                    q_all, k_all, aoT, vps = g0_tiles
                    dr_i = g0_dr
                else:
                    q_all, k_all, aoT, vps = new_group_tiles(g)
                    dr_i = [0]
                osb_p0 = [None, None]
                if prev is None:
                    pass
                else:
                    bp0 = 2 * prev["g"]
                    osb_pa = osbp.tile([128, C], f32, tag="osb")
                    osb_pb = osbp.tile([128, C], f32, tag="osb")
                    osb_p0 = [osb_pa, osb_pb]
                    for oi in range(4):
                        qk_chunk(oi, q_all, k_all, xcur["xtg"], dr_i)
                    oi = 4
                    # v half-units for THIS group's b0 and prev-b0 proj
                    # units ride the segments as extra exp-hiding fillers
                    vh = [(0, 0, 0), (0, 0, 1), (0, 1, 0), (0, 1, 1)]
                    for b_in in range(2):
                        for hp in range(H // 2):
                            pes = score_seg(b_in, hp, prev["q_all"],
                                            prev["k_all"])
                            qk_chunk(oi, q_all, k_all, xcur["xtg"], dr_i)
                            oi += 1
                            si = 4 * b_in + hp
                            if 1 <= si <= 4:
                                vb, vj, vhf = vh[si - 1]
                                v_half(g, vb, vj, vhf, xcur["xtg"],
                                       vps[vb][vj])
                            elif si >= 5:
                                nt_i, fg = [(0, 0), (0, 1),
                                            (1, 0)][si - 5]
                                proj_fg(bp0, 0, nt_i, fg, prev["aoT"],
                                        osb_p0[nt_i])
                            attnv_seg(b_in, hp, pes, prev["vps"],
                                      prev["aoT"])
                xfs_next = x_dma(g + 2) if g + 2 < NG else None
                if g < 2:
                    seed_bias_rows(q_all, k_all)
                kext(k_all)
                # tail: remaining v units + remaining proj for g-1
                if prev is None:
                    for b_in in range(2):
                        for jt in range(2):
                            v_unit(g, b_in, jt, xcur["xtg"],
                                   vps[b_in][jt])
                elif g < NG - 1:
                    # b0's v went into the segments; b1's v here
                    v_unit(g, 1, 0, xcur["xtg"], vps[1][0])
                    proj_fg(2 * prev["g"], 0, 1, 1, prev["aoT"],
                            osb_p0[1])
                    v_unit(g, 1, 1, xcur["xtg"], vps[1][1])
                    proj_b_in(prev, 1)
                else:
                    # LAST group: pull its own b0 attention into the
                    # tail, with the remaining tail work as fillers, so
                    # the epilogue only handles b1.
                    v_half(g, 1, 0, 0, xcur["xtg"], vps[1][0])
                    tail_f = [
                        lambda: v_half(g, 1, 0, 1, xcur["xtg"],
                                       vps[1][0]),
                        lambda: proj_fg(2 * prev["g"], 0, 1, 1,
                                        prev["aoT"], osb_p0[1]),
                        lambda: v_half(g, 1, 1, 0, xcur["xtg"],
                                       vps[1][1]),
                        lambda: v_half(g, 1, 1, 1, xcur["xtg"],
                                       vps[1][1]),
                    ]
                    for hp in range(H // 2):
                        pes = score_seg(0, hp, q_all, k_all)
                        tail_f[hp]()
                        attnv_seg(0, hp, pes, vps, aoT)
                    proj_b_in(prev, 1)
                prev = {"g": g, "q_all": q_all, "k_all": k_all,
                        "aoT": aoT, "vps": vps}
                xcur = xnext
                # transposes issue here, AFTER their DMA (dispatched
                # mid-group) has landed - no PE queue-head stall
                xnext = (x_transp(g + 2, xfs_next)
                         if g + 2 < NG else None)

            # ---------------- epilogue: last group's b1 ---------------
            # (b0's attention ran in the last group's tail)
            # b1 segments with proj(b0) units as fillers
            bL = 2 * prev["g"]
            osb0 = osbp.tile([128, C], f32, tag="osb")
            osb1 = osbp.tile([128, C], f32, tag="osb")
            fillers = [(0, 0, osb0), (0, 1, osb0), (1, 0, osb1),
                       (1, 1, osb1)]
            for hp in range(H // 2):
                pes = score_seg(1, hp, prev["q_all"], prev["k_all"])
                nt_i, fg, ot = fillers[hp]
                proj_fg(bL, 0, nt_i, fg, prev["aoT"], ot)
                attnv_seg(1, hp, pes, prev["vps"], prev["aoT"])
            proj_b_in(prev, 1)

    nc.compile()
    return nc


_NC_CACHE = {}


def _get_nc(b_loc):
    if b_loc not in _NC_CACHE:
        _NC_CACHE[b_loc] = build(b_loc)
    return _NC_CACHE[b_loc]


def _run(inputs, trace=False):
    from concourse.bass_utils import run_bass_kernel_spmd

    x = np.ascontiguousarray(np.asarray(inputs["x"], dtype=np.float32))
    qkv_w = np.ascontiguousarray(np.asarray(inputs["qkv_w"], dtype=np.float32))
    proj_w = np.ascontiguousarray(np.asarray(inputs["proj_w"], dtype=np.float32))
    proj_b = np.ascontiguousarray(np.asarray(inputs["proj_b"], dtype=np.float32))

    nc = _get_nc(B_LOC)
    in_maps = [
        {
            "x": np.ascontiguousarray(x[i * B_LOC:(i + 1) * B_LOC]),
            "qkv_w": qkv_w,
            "proj_w": proj_w,
            "proj_b": proj_b,
        }
        for i in range(N_CORES)
    ]
    res = run_bass_kernel_spmd(
        nc, in_maps, core_ids=list(range(N_CORES)), trace=trace)
    out = np.concatenate([r["out"] for r in res.results], axis=0)
    return out, res


def kernel(x, qkv_w, proj_w, proj_b):
    out, _ = _run({"x": x, "qkv_w": qkv_w, "proj_w": proj_w,
                   "proj_b": proj_b})
    return out
